# revision 41
# baseline (speedup 1.0000x reference)
"""Trainium2 Bass kernel for nn_DetectorKmeans (weighted-sqdist + weighted logsumexp).

dens_i = logsumexp_j( -0.5 * ||x_i - c_j||^2 / var_j + log prs_j ) - threshold

Strategy (8 NeuronCores, data-parallel over N):
  logits'_ij = a_j * (x_i . c_j) - 0.5 * a_j * ||x_i||^2 + b_j,  a_j = 1/var_j,
  b_j = -0.5 * a_j * ||c_j||^2 + log prs_j - threshold - C   (C = global shift)
  dens_i = C + log( sum_j exp(logits'_ij) )
The per-point max spread is ~40 nats for this data, far below the f32 exp
range, so one global shift C (estimated from a host-side sample) replaces the
per-point max pass.

Device layout per core (62500 pts, padded to 63488 = 496 tiles of 128):
  - "stacked" lhsT: each 128x128 SBUF block holds TWO point-tiles' features
    (64+64 rows) so DMA uses all 128 partitions (full AXI-port bandwidth).
  - MM-main vs a static block-diagonal [128, 256] centers matrix -> one PSUM
    [128,256] chunk = two tiles of logits. MM-aug (contract=5: x2 hi/lo rows
    per tile + ones row) accumulates the rank-1 x2 term and b constants.
  - batched Exp in-place on PSUM [128, 2048] blocks, batched reduce_sum on
    VectorE, final Log fused with +C via log(s * e^C).
"""

import math

import numpy as np

import concourse.bass as bass
import concourse.tile as tile
from concourse import mybir
from concourse.bass_utils import run_bass_kernel_spmd
from concourse.vector_clock import ScopedClock, VectorClock

# ---------------- problem constants (hardcoded per contract) ----------------
N, D, K = 500_000, 64, 128
NCORES = 8
PER_CORE = N // NCORES          # 62500
TILES = 496                     # padded 128-pt tiles per core
PTS_PAD = TILES * 128           # 63488
PAIRS = TILES // 2              # 248
GROUPS = 31                     # 8 pairs (2048 psum cols) per group
PAIRS_PER_GROUP = PAIRS // GROUPS

MM_DTYPE = "bf16"               # "f32" | "f32r" | "bf16"
E0 = 20.0                      # Ln-input recentering shift
AUG_BUFS = 3                   # aug pool depth (memset only on first rotation)
GPS_ASSIST_TILES = 10           # tiles per 16-tile block pre-halved on gpsimd

# ---------------- walrus 1-wait-per-instruction compat patches ----------------
_carrier_n = [0]
_orig_add_instruction = tile.TileContext._add_instruction


def _split_add_instruction(self, inst):
    si = inst.sync_info
    if si is not None and si.on_wait is not None and len(si.on_wait) > 1:
        waits = list(si.on_wait)
        for w in waits[:-1]:
            _carrier_n[0] += 1
            c = mybir.InstNoOp(name=f"waitsplit-{_carrier_n[0]}", ins=[], outs=[])
            c.engine = inst.engine
            c.sync_info = mybir.SyncInfo(on_wait=[w], on_update=[])
            _orig_add_instruction(self, c)
        inst.sync_info = mybir.SyncInfo(
            on_wait=[waits[-1]], on_update=list(si.on_update or [])
        )
    _orig_add_instruction(self, inst)


def _patched_drain_and_barrier(self, tick_clock, wait_clock):
    gc = tick_clock.global_clock
    n = len(gc)
    for p in [i for i in range(n) if gc[i] > 0]:
        sub = VectorClock([gc[i] if i == p else 0 for i in range(n)])
        d = self.nc.sync.drain()
        wait_clock.add_sem_waits(d.ins, ScopedClock({None: sub}))
    self.nc.all_engine_barrier()
    popped = self.nc._tile_sem_poison_stack.pop()
    assert popped is self._sem_poison
    self.nc.clear_and_free_semaphores(list(self.sems.allocated().values()))
    # the stock epilogue ends with a second all_engine_barrier; nothing
    # executes after the sem-clear, so it only adds ~6us of exit latency


tile.TileContext._add_instruction = _split_add_instruction
tile.TileContext._drain_and_barrier = _patched_drain_and_barrier


# ---------------- device program ----------------
_compiled = {}


def _mm_dt():
    return {
        "f32": mybir.dt.float32,
        "f32r": mybir.dt.float32r,
        "bf16": mybir.dt.bfloat16,
    }[MM_DTYPE]


def _io_np_dt():
    return np.float32 if MM_DTYPE in ("f32", "f32r") else np.dtype("bfloat16")


def _io_mybir_dt():
    if MM_DTYPE == "f32":
        return mybir.dt.float32
    if MM_DTYPE == "f32r":
        return mybir.dt.float32r
    return mybir.dt.bfloat16


def build_program():
    f32 = mybir.dt.float32
    iodt = _io_mybir_dt()
    mmdt = _mm_dt()
    nc = bass.Bass(target_bir_lowering=False)
    xmain = nc.dram_tensor("xmain", [GROUPS, 128, 1024], iodt, kind="ExternalInput").ap()
    xaug = nc.dram_tensor("xaug", [GROUPS, 5, 1024], iodt, kind="ExternalInput").ap()
    rmain = nc.dram_tensor("rmain", [128, 256], iodt, kind="ExternalInput").ap()
    raug = nc.dram_tensor("raug", [128, 256], iodt, kind="ExternalInput").ap()
    addc = nc.dram_tensor("addc", [128, 1], f32, kind="ExternalInput").ap()
    dens = nc.dram_tensor("dens", [128, TILES], f32, kind="ExternalOutput").ap()

    with tile.TileContext(nc) as tc:
        with (
            tc.tile_pool(name="consts", bufs=1) as cpool,
            tc.tile_pool(name="xp", bufs=3) as xpool,
            tc.tile_pool(name="ap", bufs=AUG_BUFS) as apool,
            tc.tile_pool(name="ps", bufs=2, space="PSUM") as pspool,
            tc.tile_pool(name="es", bufs=3) as espool,
            tc.tile_pool(name="hv", bufs=3) as hvpool,
            tc.tile_pool(name="acc", bufs=1) as accpool,
        ):
            rm = cpool.tile([128, 256], iodt)
            nc.sync.dma_start(rm[:], rmain[:])
            ra = cpool.tile([128, 256], iodt)
            nc.sync.dma_start(ra[:], raug[:])
            adc = cpool.tile([128, 1], f32)
            s_sb = accpool.tile([128, TILES], f32)
            ln_sb = accpool.tile([128, TILES], f32)
            out_sb = accpool.tile([128, TILES], f32)

            cast = (lambda ap: ap) if iodt == mmdt else (lambda ap: ap.bitcast(mmdt))
            rm_mm = cast(rm[:])
            ra_mm = cast(ra[:])

            def tail_chunk(c0, c1):
                # dens = C + log(s) = log(s * e^E0) + (C - E0); the e^E0 shift
                # keeps the Ln input inside the spline's accurate domain
                nc.scalar.activation(
                    ln_sb[:, c0:c1],
                    s_sb[:, c0:c1],
                    mybir.ActivationFunctionType.Ln,
                    bias=0.0,
                    scale=float(math.exp(E0)),
                )
                nc.scalar.activation(
                    out_sb[:, c0:c1],
                    ln_sb[:, c0:c1],
                    mybir.ActivationFunctionType.Identity,
                    bias=adc[:],
                    scale=1.0,
                )
                nc.sync.dma_start(dens[:, c0:c1], out_sb[:, c0:c1])

            HALF_TILES = GPS_ASSIST_TILES  # tiles per block pre-halved on gpsimd
            for g in range(GROUPS):
                xg = xpool.tile([128, 1024], iodt)
                nc.sync.dma_start(xg[:], xmain[g])
                # aug lhsT padded to K=128: rows 5:128 are zeros (their rhs rows
                # are zero too) -- K=5 matmuls are ~3x slower than K=128.
                # Pool slots rotate every `bufs` groups and keep their zeros, so
                # only the first `bufs` groups memset.
                ag = apool.tile([128, 1024], iodt)
                if g < AUG_BUFS:
                    nc.gpsimd.memset(ag[:], 0.0)
                nc.sync.dma_start(ag[0:5, :], xaug[g])
                if g == 2:
                    nc.sync.dma_start(adc[:], addc[:])
                pb = pspool.tile([128, 2048], f32)
                for p in range(PAIRS_PER_GROUP):
                    out_ap = pb[:, p * 256 : (p + 1) * 256]
                    nc.tensor.matmul(
                        out_ap,
                        cast(xg[:, p * 128 : (p + 1) * 128]),
                        rm_mm,
                        start=True,
                        stop=False,
                    )
                    nc.tensor.matmul(
                        out_ap,
                        cast(ag[:, p * 128 : (p + 1) * 128]),
                        ra_mm,
                        start=False,
                        stop=True,
                    )
                # exp to SBUF staging so the psum block frees for the next MMs
                # and ACT/DVE pipeline independently
                eg = espool.tile([128, 2048], mybir.dt.bfloat16)
                nc.scalar.activation(
                    eg[:], pb[:], mybir.ActivationFunctionType.Exp, bias=0.0, scale=1.0
                )
                egv = eg[:].rearrange("p (t c) -> p t c", c=128)
                if HALF_TILES:
                    # gpsimd pre-halves the first HALF_TILES tiles (otherwise
                    # idle engine), shrinking the DVE reduce's element count
                    hv = hvpool.tile([128, HALF_TILES * 64], f32)
                    hvv = hv[:].rearrange("p (t c) -> p t c", c=64)
                    nc.gpsimd.tensor_add(
                        hvv,
                        egv[:, 0:HALF_TILES, 0:64],
                        egv[:, 0:HALF_TILES, 64:128],
                    )
                    # non-halved reduce first: it does not depend on gpsimd, so
                    # the in-order DVE works while gpsimd halves
                    nc.vector.reduce_sum(
                        s_sb[:, g * 16 + HALF_TILES : (g + 1) * 16],
                        egv[:, HALF_TILES:16, :],
                        axis=mybir.AxisListType.X,
                    )
                    nc.vector.reduce_sum(
                        s_sb[:, g * 16 : g * 16 + HALF_TILES],
                        hvv,
                        axis=mybir.AxisListType.X,
                    )
                else:
                    nc.vector.reduce_sum(
                        s_sb[:, g * 16 : (g + 1) * 16], egv, axis=mybir.AxisListType.X
                    )
                if g % 8 == 7:
                    tail_chunk((g - 7) * 16, (g + 1) * 16)
            tail_chunk(384, TILES)
    return nc


# ---------------- host side ----------------
def _prepare(X, centers, vars_, prs, threshold):
    X = np.asarray(X, np.float32)
    centers = np.asarray(centers, np.float32)
    vars_ = np.asarray(vars_, np.float32)
    prs = np.asarray(prs, np.float32)
    thr = float(np.asarray(threshold).reshape(-1)[0])

    a = (1.0 / vars_).astype(np.float32)                       # [K]
    ac = (centers * a[:, None]).astype(np.float32)             # [K, D]
    c2 = (centers.astype(np.float64) ** 2).sum(1)
    b = (-0.5 * a.astype(np.float64) * c2 + np.log(prs.astype(np.float64)) - thr)

    # global shift C from a host-side sample (spread of per-point maxima is
    # ~40 nats for this distribution; +-30 nats of slack either way)
    xs = X[:: max(1, N // 2048)][:2048].astype(np.float64)
    ls = (
        a[None, :] * (xs @ centers.T.astype(np.float64))
        - 0.5 * a[None, :] * (xs**2).sum(1)[:, None]
        + b[None, :]
    )
    C = float(ls.max())
    bbar = (b - C).astype(np.float32)

    iodt = _io_np_dt()

    # static rhs blocks
    rmain = np.zeros((128, 256), np.float32)
    rmain[0:64, 0:128] = ac.T
    rmain[64:128, 128:256] = ac.T
    raug = np.zeros((128, 256), np.float32)
    raug[0, 0:128] = -0.5 * a
    raug[1, 0:128] = -0.5 * a
    raug[2, 128:256] = -0.5 * a
    raug[3, 128:256] = -0.5 * a
    raug[4, 0:128] = bbar
    raug[4, 128:256] = bbar

    in_maps = []
    for c in range(NCORES):
        xc = np.zeros((PTS_PAD, D), np.float32)
        xc[:PER_CORE] = X[c * PER_CORE : (c + 1) * PER_CORE]
        # stacked lhsT stripes [GROUPS, 128, 1024]
        xm = (
            xc.reshape(PAIRS, 2, 128, D)
            .transpose(0, 1, 3, 2)
            .reshape(GROUPS, PAIRS_PER_GROUP, 128, 128)
            .transpose(0, 2, 1, 3)
            .reshape(GROUPS, 128, 1024)
        )
        # x2 rows with hi/lo (bf16-exact hi, small lo) split
        x2 = (xc.astype(np.float64) ** 2).sum(1).astype(np.float32)
        hi = x2.astype(np.dtype("bfloat16")).astype(np.float32)
        lo = x2 - hi
        aug = np.zeros((PAIRS, 5, 128), np.float32)
        x2p = x2.reshape(PAIRS, 2, 128)
        hip = hi.reshape(PAIRS, 2, 128)
        lop = lo.reshape(PAIRS, 2, 128)
        aug[:, 0] = hip[:, 0]
        aug[:, 1] = lop[:, 0]
        aug[:, 2] = hip[:, 1]
        aug[:, 3] = lop[:, 1]
        aug[:, 4] = 1.0
        xa = (
            aug.reshape(GROUPS, PAIRS_PER_GROUP, 5, 128)
            .transpose(0, 2, 1, 3)
            .reshape(GROUPS, 5, 1024)
        )
        in_maps.append(
            {
                "xmain": np.ascontiguousarray(xm).astype(iodt),
                "xaug": np.ascontiguousarray(xa).astype(iodt),
                "rmain": rmain.astype(iodt),
                "raug": raug.astype(iodt),
                "addc": np.full((128, 1), C - E0, np.float32),
            }
        )
    return in_maps, C


_last_result = {}


def kernel(X, centers, vars_, prs, threshold):
    in_maps, C = _prepare(X, centers, vars_, prs, threshold)
    if "nc" not in _compiled:
        _compiled["nc"] = build_program()
    nc = _compiled["nc"]
    trace = _last_result.get("want_trace", False)
    r = run_bass_kernel_spmd(nc, in_maps, list(range(NCORES)), trace=trace)
    _last_result["r"] = r
    outs = []
    for c in range(NCORES):
        d = np.asarray(r.results[c]["dens"])  # [128, TILES]
        outs.append(d.T.reshape(-1)[:PER_CORE])
    return np.concatenate(outs).astype(np.float32)
